# revision 20
# baseline (speedup 1.0000x reference)
"""Trainium2 Bass kernel for nn_Ext_relLayer (GNN message passing).

Strategy (edge-parallel, sharded by destination-node ownership):
  * The per-edge masked linear is decomposed by linearity:
      msg[e] = P_{inv}[rel[e]] + rel_emb[src[e]] @ W_{inv,B}.T
    where P_i = metarel_emb @ W_{i,A}.T + b_i  (A = first D cols, B = last D).
    The W_B matmul is pushed past the segment-sum, so the edge phase is pure
    gather + masked segment-sum:
      agg[n] = SP[n] + S0[n] @ W_IB.T + S1[n] @ W_OB.T
      SP[n] = sum_{e->n} P_{r2[e]},  Si[n] = sum_{e->n, inv=i} rel_emb[src[e]]
  * SP and the in-degree counts are computed without any per-edge work: the
    host bincounts (node, rel2) pairs into a count matrix C (pure index
    preprocessing), and the device computes SP = C @ P and cnt = C @ 1 as
    dense PE matmuls against the on-chip P table.
  * 50000 nodes are assigned to 8 cores x 49 tiles x 128 slots, balancing
    per-tile edge counts per src-half. Each edge lives on the core owning
    its dst. Per core, edges are ordered by (src_half, dst_tile); groups are
    padded to a multiple of 128 (sizes maxed over cores -> one static SPMD
    program).
  * Edge phase per 128-edge tile: dma_gather 512B rows v = rel_emb[src]
    stored as [bf16(x) | bf16(x - hi)] pairs; two masked one-hots
    (dst_slot_inv0/1 == iota) scatter-add v into S0/S1 PSUM columns via
    TensorE (hi+lo matmuls accumulate -> exact to ~2^-17).
  * Post phase per node tile: transpose S0/S1 on TensorE, apply W_IB/W_OB,
    add SP, scale by 1/max(cnt,1), add rel_emb @ W_S.T + b_S. meta_new is a
    tiny dense matmul done on every core (host takes core 0's copy).
"""
import sys
import numpy as np

sys.path.insert(0, '/opt/trn_rl_repo')

import ml_dtypes  # noqa: E402

N_NODES, N_EDGES, N_REL, D = 50000, 640000, 500, 128
R2 = 2 * N_REL               # (rel, inv) alphabet
N_CORES = 8
NODE_TILES = 49              # node tiles per core
SLOTS = NODE_TILES * 128     # node slots per core
HALF = 25000                 # src table split for int16 gather indices
PAD_SLOT = 255.0
BATCH = 6144                 # edges per dma_gather call


# ---------------------------------------------------------------- host prep

def _assign_nodes(src, dst):
    """Assign nodes to (core, tile, slot) balancing per-(bin, src_half) edge
    counts so the cross-core maxed group sizes stay close to the mean."""
    half = (src >= HALF).astype(np.int64)
    deg0 = np.bincount(dst[half == 0], minlength=N_NODES)
    deg1 = np.bincount(dst[half == 1], minlength=N_NODES)
    deg = deg0 + deg1
    order = np.argsort(-deg, kind="stable")
    n_bins = N_CORES * NODE_TILES
    load = np.zeros((n_bins, 2), dtype=np.int64)
    fill = np.zeros(n_bins, dtype=np.int32)
    node_bin = np.empty(N_NODES, dtype=np.int32)
    INF = np.int64(1) << 50
    for n in order:
        d0, d1 = deg0[n], deg1[n]
        # quadratic potential: prefer the bin where the load stays most even
        cost = (load[:, 0] + d0) ** 2 + (load[:, 1] + d1) ** 2
        cost = np.where(fill < 128, cost, INF)
        b = int(np.argmin(cost))
        node_bin[n] = b
        load[b, 0] += d0
        load[b, 1] += d1
        fill[b] += 1
    node_core = (node_bin // NODE_TILES).astype(np.int32)
    node_tile = (node_bin % NODE_TILES).astype(np.int32)
    node_slot = np.empty(N_NODES, dtype=np.int32)
    fill[:] = 0
    for n in order:
        b = node_bin[n]
        node_slot[n] = fill[b]
        fill[b] += 1
    perm = -np.ones((N_CORES, SLOTS), dtype=np.int64)
    perm[node_core, node_tile * 128 + node_slot] = np.arange(N_NODES)
    return node_core, node_tile, node_slot, perm


def _build_schedule(src, dst, rel, inv):
    node_core, node_tile, node_slot, perm = _assign_nodes(src, dst)
    core = node_core[dst]
    r2 = rel + N_REL * inv
    half = (src >= HALF).astype(np.int64)
    gidx_all = np.where(half == 0, src, src - HALF)

    tile_of = node_tile[dst]
    key = (half * NODE_TILES + tile_of).astype(np.int64)
    n_groups = 2 * NODE_TILES

    counts = np.zeros((N_CORES, n_groups), dtype=np.int64)
    for c in range(N_CORES):
        counts[c] = np.bincount(key[core == c], minlength=n_groups)
    # per-half padded sizes (shared across cores); tiles may straddle
    # adjacent dst_tile groups within a half
    half_sizes = counts.reshape(N_CORES, 2, NODE_TILES).sum(axis=2).max(axis=0)
    half_pad = (half_sizes + 127) // 128 * 128
    epad = int(half_pad.sum())
    half_split = int(half_pad[0])
    hoff = np.array([0, half_split])

    gidx = np.zeros((N_CORES, epad), dtype=np.int16)
    dslot0 = np.full((N_CORES, epad), PAD_SLOT, dtype=np.float32)
    dslot1 = np.full((N_CORES, epad), PAD_SLOT, dtype=np.float32)
    # per-edge dst_tile id (for boundary-tile session structure), -1 for pads
    tileid = np.full((N_CORES, epad), -1, dtype=np.int32)
    # count matrix per core: C_T[r2, slot_global], stored chunked for DMA:
    # ct8[p, ch, col] = C_T[ch*125 + p, col]
    ct8 = np.zeros((N_CORES, 125, 8, SLOTS), dtype=ml_dtypes.bfloat16)
    for c in range(N_CORES):
        m = core == c
        k = key[m]
        o = np.argsort(k, kind="stable")
        ks = k[o]
        h_of = ks // NODE_TILES
        pos = np.arange(len(ks)) - np.searchsorted(ks, h_of * NODE_TILES)
        at = hoff[h_of] + pos
        gidx[c, at] = gidx_all[m][o].astype(np.int16)
        sl = node_slot[dst[m][o]].astype(np.float32)
        iv = inv[m][o]
        dslot0[c, at] = np.where(iv == 0, sl, PAD_SLOT)
        dslot1[c, at] = np.where(iv == 1, sl, PAD_SLOT)
        tileid[c, at] = (ks % NODE_TILES).astype(np.int32)
        colg = (node_tile[dst[m]] * 128 + node_slot[dst[m]]).astype(np.int64)
        flat = np.bincount(colg * R2 + r2[m], minlength=SLOTS * R2)
        ct = flat.reshape(SLOTS, R2).T.astype(np.float32)  # [R2, SLOTS]
        ct8[c] = ct.reshape(8, 125, SLOTS).transpose(1, 0, 2).astype(
            ml_dtypes.bfloat16)

    # static tile plan: for each 128-edge tile and each dst_tile it can
    # touch on ANY core, one (matmul-pair, compare) slot. A dst_tile's
    # session spans [first_tile, last_tile] maxed over cores.
    n_tiles = epad // 128
    touch = np.zeros((n_tiles, NODE_TILES), dtype=bool)
    for c in range(N_CORES):
        tv = tileid[c].reshape(n_tiles, 128)
        for t in range(NODE_TILES):
            touch[:, t] |= (tv == t).any(axis=1)
    # sessions must be contiguous tile ranges; fill gaps (cheap)
    for t in range(NODE_TILES):
        w = np.nonzero(touch[:, t])[0]
        for h in range(2):
            lo_t, hi_t = hoff[h] // 128, (hoff[h] + half_pad[h]) // 128
            wh = w[(w >= lo_t) & (w < hi_t)]
            if len(wh):
                touch[wh[0]:wh[-1] + 1, t] = True

    # per-core dspair entries exist per (tile, touched dst_tile): the host
    # rewrites slots relative to each touched dst_tile (255 when the edge
    # belongs to a different dst_tile)
    return dict(perm=perm, epad=epad, half_split=half_split, gidx=gidx,
                dslot0=dslot0, dslot1=dslot1, tileid=tileid, touch=touch,
                ct8=ct8)


def _pair_plan(touch, half_split, epad):
    """Static (tile, dst_tile) pair list + session open/close markers."""
    n_tiles = epad // 128
    hs_tile = half_split // 128
    pairs = []
    for tile in range(n_tiles):
        for t in np.nonzero(touch[tile])[0]:
            pairs.append((tile, int(t)))
    first = {}
    last = {}
    for tile, t in pairs:
        h = 0 if tile < hs_tile else 1
        if (h, t) not in first:
            first[(h, t)] = tile
        last[(h, t)] = tile
    return pairs, first, last


def _build_dspair(dslot0, dslot1, tileid, pairs):
    """[128, 2*n_pairs] f32: per pair, slots masked to that dst_tile."""
    n_pairs = len(pairs)
    d0 = dslot0.reshape(-1, 128)
    d1 = dslot1.reshape(-1, 128)
    tv = tileid.reshape(-1, 128)
    out = np.full((128, 2 * n_pairs), PAD_SLOT, dtype=np.float32)
    for p, (tile, t) in enumerate(pairs):
        m = tv[tile] == t
        out[:, 2 * p] = np.where(m, d0[tile], PAD_SLOT)
        out[:, 2 * p + 1] = np.where(m, d1[tile], PAD_SLOT)
    return out


def _hilo(a):
    hi = a.astype(ml_dtypes.bfloat16)
    lo = (a - hi.astype(np.float32)).astype(ml_dtypes.bfloat16)
    return np.concatenate([hi, lo], axis=-1).copy()


def _wrap16(a):
    return np.tile(a.reshape(-1, 16).T, (8, 1)).copy()


def _slot_layout(a):
    return a.reshape(-1, 128).T.copy()


# ------------------------------------------------------------ bass program

def _build_program(pairs, first, last, epad, half_split):
    import os
    from concourse import bass, bacc, tile
    mybir = bass.mybir
    f32, bf16, i16 = mybir.dt.float32, mybir.dt.bfloat16, mybir.dt.int16
    add = mybir.AluOpType.add

    nc = bacc.Bacc(None, target_bir_lowering=False)
    dp = nc.declare_dram_parameter
    vtab = dp("vtab", [N_NODES, 2 * D], bf16, isOutput=False)
    metarelT = dp("metarelT", [D, 512], f32, isOutput=False)
    wia_t = dp("wia_t", [D, D], f32, isOutput=False)
    woa_t = dp("woa_t", [D, D], f32, isOutput=False)
    wib_t = dp("wib_t", [D, D], f32, isOutput=False)
    wob_t = dp("wob_t", [D, D], f32, isOutput=False)
    ws_t = dp("ws_t", [D, D], f32, isOutput=False)
    wm_t = dp("wm_t", [D, D], f32, isOutput=False)
    bias_i = dp("bias_i", [128, D], f32, isOutput=False)
    bias_o = dp("bias_o", [128, D], f32, isOutput=False)
    bias_s = dp("bias_s", [128, D], f32, isOutput=False)
    bias_m = dp("bias_m", [128, D], f32, isOutput=False)
    n_pairs = len(pairs)
    iota2 = dp("iota2", [128, 256], f32, isOutput=False)
    gidx = dp("gidx", [128, epad // 16], i16, isOutput=False)
    dspair = dp("dspair", [128, 2 * n_pairs], f32, isOutput=False)
    ct8 = dp("ct8", [125, 8 * SLOTS], bf16, isOutput=False)
    relembT = dp("relembT", [D, SLOTS], f32, isOutput=False)
    out_rel = dp("out_rel", [SLOTS, D], f32, isOutput=True)
    out_meta = dp("out_meta", [N_REL, D], f32, isOutput=True)

    n_tiles = epad // 128

    with tile.TileContext(nc) as tc:
        with tc.tile_pool(name="const", bufs=1) as cpool, \
             tc.tile_pool(name="acc", bufs=1) as apool, \
             tc.tile_pool(name="buf", bufs=3) as bpool, \
             tc.tile_pool(name="oh", bufs=6) as ohpool, \
             tc.tile_pool(name="work", bufs=3) as wpool, \
             tc.tile_pool(name="stage", bufs=3) as spool, \
             tc.tile_pool(name="ct", bufs=3) as ctpool, \
             tc.tile_pool(name="ps", bufs=4, space="PSUM") as pspool, \
             tc.tile_pool(name="ps2", bufs=2, space="PSUM") as ps2pool, \
             tc.tile_pool(name="psx", bufs=2, space="PSUM") as psxpool:

            # ---- constants to SBUF
            def ld(nm, shape, dt_, src):
                t_ = cpool.tile(shape, dt_, tag=nm, name=nm)
                nc.sync.dma_start(out=t_[:], in_=src[:])
                return t_

            metarelT_sb = ld("metarelT_sb", [D, 512], f32, metarelT)
            wia_sb = ld("wia_sb", [D, D], f32, wia_t)
            woa_sb = ld("woa_sb", [D, D], f32, woa_t)
            wib_sb = ld("wib_sb", [D, D], f32, wib_t)
            wob_sb = ld("wob_sb", [D, D], f32, wob_t)
            ws_sb = ld("ws_sb", [D, D], f32, ws_t)
            wm_sb = ld("wm_sb", [D, D], f32, wm_t)
            bias_i_sb = ld("bias_i_sb", [128, D], f32, bias_i)
            bias_o_sb = ld("bias_o_sb", [128, D], f32, bias_o)
            bias_s_sb = ld("bias_s_sb", [128, D], f32, bias_s)
            bias_m_sb = ld("bias_m_sb", [128, D], f32, bias_m)
            iota2_sb = ld("iota2_sb", [128, 256], f32, iota2)
            gidx_sb = cpool.tile([128, epad // 16], i16, tag="gidx_sb",
                                 name="gidx_sb")
            c0 = min(BATCH // 16, epad // 16)
            nc.sync.dma_start(out=gidx_sb[:, :c0], in_=gidx[:, :c0])
            nc.sync.dma_start(out=gidx_sb[:, c0:], in_=gidx[:, c0:])
            dspair_sb = ld("dspair_sb", [128, 2 * n_pairs], f32, dspair)

            # ---- phase 0: P tables -> SBUF rhs [125, 8, 257] = [hi|lo|1]
            p_rhs = cpool.tile([125, 8, 2 * D + 1], bf16, tag="p_rhs",
                               name="p_rhs")
            nc.vector.memset(p_rhs[:], 1.0)  # ones column (rest overwritten)
            for ti, (w_sb, b_sb) in enumerate(((wia_sb, bias_i_sb),
                                               (woa_sb, bias_o_sb))):
                for ch4 in range(4):
                    rows = 125
                    col0 = ch4 * 125
                    ch = ti * 4 + ch4
                    psp = psxpool.tile([128, 2 * D + 1], f32, tag="psx",
                                       name="psp")
                    nc.tensor.matmul(psp[:rows, 0:D],
                                     metarelT_sb[:, col0:col0 + rows],
                                     w_sb[:], start=True, stop=True)
                    pf = wpool.tile([128, D], f32, tag="pf", name="pf")
                    nc.vector.tensor_tensor(pf[:rows], psp[:rows, 0:D],
                                            b_sb[:rows], add)
                    nc.vector.tensor_copy(p_rhs[:, ch, 0:D], pf[:rows])
                    phi32 = wpool.tile([128, D], f32, tag="phi32",
                                       name="phi32")
                    nc.vector.tensor_copy(phi32[:rows], p_rhs[:, ch, 0:D])
                    nc.vector.tensor_tensor(p_rhs[:, ch, D:2 * D], pf[:rows],
                                            phi32[:rows],
                                            mybir.AluOpType.subtract)

            # ---- accumulators
            sp_acc = [apool.tile([128, D], f32, tag=f"sp{t}", name=f"sp{t}")
                      for t in range(NODE_TILES)]
            s0_acc = [apool.tile([128, D], f32, tag=f"s0{t}", name=f"s0{t}")
                      for t in range(NODE_TILES)]  # [fin, nodes] layout
            s1_acc = [apool.tile([128, D], f32, tag=f"s1{t}", name=f"s1{t}")
                      for t in range(NODE_TILES)]  # [fin, nodes] layout
            cnt_acc = [apool.tile([128, 1], f32, tag=f"cnt{t}",
                                  name=f"cnt{t}")
                       for t in range(NODE_TILES)]

            # ---- C phase (emitted lazily per node tile):
            # SP = C @ (P_hi + P_lo), cnt = C @ 1
            ct8_3d = ct8[:, :].rearrange("p (c s) -> p c s", c=8)

            def emit_cphase(t):
                ctile = ctpool.tile([125, 8, 128], bf16, tag="ctile",
                                    name="ctile")
                nc.sync.dma_start(
                    out=ctile[:],
                    in_=ct8_3d[:, :, t * 128:(t + 1) * 128])
                psC = psxpool.tile([128, 2 * D + 1], f32, tag="psx",
                                   name="psC")
                for ch in range(8):
                    nc.tensor.matmul(psC[:], ctile[:, ch, :],
                                     p_rhs[:, ch, :],
                                     start=(ch == 0), stop=(ch == 7))
                nc.vector.tensor_copy(sp_acc[t][:], psC[:, 0:D])
                nc.vector.tensor_tensor(sp_acc[t][:], sp_acc[t][:],
                                        psC[:, D:2 * D], add)
                nc.vector.tensor_scalar(cnt_acc[t][:], psC[:, 2 * D:2 * D + 1],
                                        1.0, None, op0=mybir.AluOpType.max)
                nc.vector.reciprocal(cnt_acc[t][:], cnt_acc[t][:])

            # ---- edge phase
            CH = BATCH // 128
            batches = []
            hs_tile = half_split // 128
            for lo, hi_ in ((0, hs_tile), (hs_tile, n_tiles)):
                s = lo
                while s < hi_:
                    n = min(CH, hi_ - s)
                    batches.append((s, n))
                    s += n
            tile2batch = {}
            for bi, (s, n) in enumerate(batches):
                for t_ in range(s, s + n):
                    tile2batch[t_] = (bi, t_ - s)

            bufs = {}
            # ---- meta_new (emitted early; independent of edges)
            for ch4 in range(4):
                rows = 125
                col0 = ch4 * 125
                psm = psxpool.tile([128, 2 * D + 1], f32, tag="psx",
                                   name="psm")
                nc.tensor.matmul(psm[:rows, 0:D],
                                 metarelT_sb[:, col0:col0 + rows],
                                 wm_sb[:], start=True, stop=True)
                om = spool.tile([128, D], f32, tag="om", name="om")
                nc.vector.tensor_tensor(om[:rows], psm[:rows, 0:D],
                                        bias_m_sb[:rows], add)
                nc.sync.dma_start(out=out_meta[col0:col0 + rows, :],
                                  in_=om[:rows])

            def emit_post(t):
                ps2 = ps2pool.tile([128, D], f32, tag="ps2", name="ps2")
                nc.tensor.matmul(ps2[:], s0_acc[t][:], wib_sb[:],
                                 start=True, stop=False)
                nc.tensor.matmul(ps2[:], s1_acc[t][:], wob_sb[:],
                                 start=False, stop=True)
                ret = spool.tile([128, D], f32, tag="ret", name="ret")
                nc.sync.dma_start(out=ret[:],
                                  in_=relembT[:, t * 128:(t + 1) * 128])
                ps3 = ps2pool.tile([128, D], f32, tag="ps2", name="ps3")
                nc.tensor.matmul(ps3[:], ret[:], ws_sb[:],
                                 start=True, stop=True)
                o1 = spool.tile([128, D], f32, tag="o1", name="o1")
                nc.vector.tensor_tensor(o1[:], ps2[:], sp_acc[t][:], add)
                nc.vector.tensor_tensor(o1[:], o1[:],
                                        cnt_acc[t][:].broadcast_to([128, D]),
                                        mybir.AluOpType.mult)
                nc.vector.tensor_tensor(o1[:], o1[:], ps3[:], add)
                nc.vector.tensor_tensor(o1[:], o1[:], bias_s_sb[:], add)
                nc.sync.dma_start(out=out_rel[t * 128:(t + 1) * 128, :],
                                  in_=o1[:])

            ps_live = {}
            for p_idx, (t_idx, t) in enumerate(pairs):
                h = 0 if t_idx < hs_tile else 1
                is_first = first[(h, t)] == t_idx
                is_last = last[(h, t)] == t_idx
                if h == 1 and is_first:
                    emit_cphase(t)
                bi, c = tile2batch[t_idx]
                if c == 0 and bi not in bufs:
                    s, n = batches[bi]
                    buf = bpool.tile([128, CH, 2 * D], bf16, tag="gbuf",
                                     name="gbuf")
                    e0, ne = s * 128, n * 128
                    vbase = 0 if t_idx < hs_tile else HALF
                    nc.gpsimd.dma_gather(
                        buf[:, :n], vtab[vbase:vbase + HALF, :],
                        gidx_sb[:, e0 // 16:(e0 + ne) // 16], ne, ne,
                        2 * D, single_packet=False)
                    bufs[bi] = buf
                buf = bufs[bi]

                oh = ohpool.tile([128, 2, 128], bf16, tag="oh", name="oh")
                nc.vector.tensor_tensor(
                    oh[:], iota2_sb[:].rearrange("p (b j) -> p b j", b=2),
                    dspair_sb[:, 2 * p_idx:2 * p_idx + 2]
                    .broadcast_to([128, 2, 128]),
                    mybir.AluOpType.is_equal)
                if is_first:
                    ps_live[t] = pspool.tile([128, 256], f32, tag="ps",
                                             name="ps")
                ps = ps_live[t]
                nc.tensor.matmul(ps[:], buf[:, c, 0:D], oh[:],
                                 start=is_first, stop=False)
                nc.tensor.matmul(ps[:], buf[:, c, D:2 * D], oh[:],
                                 start=False, stop=is_last)

                if is_last:
                    del ps_live[t]
                    if h == 0:
                        nc.vector.tensor_copy(s0_acc[t][:], ps[:, 0:D])
                        nc.vector.tensor_copy(s1_acc[t][:], ps[:, D:2 * D])
                    else:
                        nc.vector.tensor_tensor(s0_acc[t][:], s0_acc[t][:],
                                                ps[:, 0:D], add)
                        nc.vector.tensor_tensor(s1_acc[t][:], s1_acc[t][:],
                                                ps[:, D:2 * D], add)
                        emit_post(t)


    nc.finalize()
    return nc


_CACHE = {}


def kernel(rel_emb, metarel_emb, src, dst, rel, inv,
           W_I_w, W_I_b, W_O_w, W_O_b, W_S_w, W_S_b, W_M_w, W_M_b):
    from concourse.bass_utils import run_bass_kernel_spmd

    rel_emb = np.asarray(rel_emb, dtype=np.float32)
    metarel_emb = np.asarray(metarel_emb, dtype=np.float32)
    src = np.asarray(src).astype(np.int64)
    dst = np.asarray(dst).astype(np.int64)
    rel = np.asarray(rel).astype(np.int64)
    inv = np.asarray(inv).astype(np.int64)
    W_I_w = np.asarray(W_I_w, dtype=np.float32)
    W_O_w = np.asarray(W_O_w, dtype=np.float32)
    W_S_w = np.asarray(W_S_w, dtype=np.float32)
    W_M_w = np.asarray(W_M_w, dtype=np.float32)

    sched = _build_schedule(src, dst, rel, inv)
    pairs, first, last = _pair_plan(sched['touch'], sched['half_split'],
                                    sched['epad'])
    key = (int(sched['epad']), int(sched['half_split']),
           sched['touch'].tobytes())
    if key not in _CACHE:
        _CACHE[key] = _build_program(pairs, first, last, sched['epad'],
                                     sched['half_split'])
    nc = _CACHE[key]

    vtab = _hilo(rel_emb)
    metarelT = np.zeros((D, 512), np.float32)
    metarelT[:, :N_REL] = metarel_emb.T
    bcast = lambda b: np.tile(np.asarray(b, np.float32)[None, :],
                              (128, 1)).copy()

    shared = {
        "vtab": vtab,
        "metarelT": metarelT,
        "wia_t": W_I_w[:, :D].T.copy(), "woa_t": W_O_w[:, :D].T.copy(),
        "wib_t": W_I_w[:, D:].T.copy(), "wob_t": W_O_w[:, D:].T.copy(),
        "ws_t": W_S_w.T.copy(), "wm_t": W_M_w.T.copy(),
        "bias_i": bcast(W_I_b), "bias_o": bcast(W_O_b),
        "bias_s": bcast(W_S_b), "bias_m": bcast(W_M_b),
        "iota2": np.tile(np.arange(128, dtype=np.float32), (128, 2)).copy(),
    }
    perm = sched['perm']
    in_maps = []
    for c in range(N_CORES):
        rows = np.where(perm[c] >= 0, perm[c], 0)
        relembT = rel_emb[rows].T.copy()
        dspair = _build_dspair(sched['dslot0'][c], sched['dslot1'][c],
                               sched['tileid'][c], pairs)
        in_maps.append(dict(shared,
                            gidx=_wrap16(sched['gidx'][c]),
                            dspair=dspair,
                            ct8=sched['ct8'][c].reshape(125, 8 * SLOTS).copy(),
                            relembT=relembT))

    try:
        res = run_bass_kernel_spmd(nc, in_maps, list(range(N_CORES)))
    except Exception:
        # transient NRT device errors (e.g. a previous crashed process left
        # a core wedged) usually clear on retry
        res = run_bass_kernel_spmd(nc, in_maps, list(range(N_CORES)))
    globals()['_last_results'] = res

    rel_new = np.zeros((N_NODES, D), np.float32)
    for c in range(N_CORES):
        valid = perm[c] >= 0
        rel_new[perm[c][valid]] = res.results[c]["out_rel"][valid]
    meta_new = res.results[0]["out_meta"]
    return rel_new, meta_new


# revision 21
# speedup vs baseline: 1.0183x; 1.0183x over previous
"""Trainium2 Bass kernel for nn_Ext_relLayer (GNN message passing).

Strategy (edge-parallel, sharded by destination-node ownership):
  * The per-edge masked linear is decomposed by linearity:
      msg[e] = P_{inv}[rel[e]] + rel_emb[src[e]] @ W_{inv,B}.T
    where P_i = metarel_emb @ W_{i,A}.T + b_i  (A = first D cols, B = last D).
    The W_B matmul is pushed past the segment-sum, so the edge phase is pure
    gather + masked segment-sum:
      agg[n] = SP[n] + S0[n] @ W_IB.T + S1[n] @ W_OB.T
      SP[n] = sum_{e->n} P_{r2[e]},  Si[n] = sum_{e->n, inv=i} rel_emb[src[e]]
  * SP and the in-degree counts are computed without any per-edge work: the
    host bincounts (node, rel2) pairs into a count matrix C (pure index
    preprocessing), and the device computes SP = C @ P and cnt = C @ 1 as
    dense PE matmuls against the on-chip P table.
  * 50000 nodes are assigned to 8 cores x 49 tiles x 128 slots, balancing
    per-tile edge counts per src-half. Each edge lives on the core owning
    its dst. Per core, edges are ordered by (src_half, dst_tile); groups are
    padded to a multiple of 128 (sizes maxed over cores -> one static SPMD
    program).
  * Edge phase per 128-edge tile: dma_gather 512B rows v = rel_emb[src]
    stored as [bf16(x) | bf16(x - hi)] pairs; two masked one-hots
    (dst_slot_inv0/1 == iota) scatter-add v into S0/S1 PSUM columns via
    TensorE (hi+lo matmuls accumulate -> exact to ~2^-17).
  * Post phase per node tile: transpose S0/S1 on TensorE, apply W_IB/W_OB,
    add SP, scale by 1/max(cnt,1), add rel_emb @ W_S.T + b_S. meta_new is a
    tiny dense matmul done on every core (host takes core 0's copy).
"""
import sys
import numpy as np

sys.path.insert(0, '/opt/trn_rl_repo')

import ml_dtypes  # noqa: E402

N_NODES, N_EDGES, N_REL, D = 50000, 640000, 500, 128
R2 = 2 * N_REL               # (rel, inv) alphabet
N_CORES = 8
NODE_TILES = 49              # node tiles per core
SLOTS = NODE_TILES * 128     # node slots per core
HALF = 25000                 # src table split for int16 gather indices
PAD_SLOT = 255.0
BATCH = 4096                 # edges per dma_gather call


# ---------------------------------------------------------------- host prep

def _assign_nodes(src, dst):
    """Assign nodes to (core, tile, slot) balancing per-(bin, src_half) edge
    counts so the cross-core maxed group sizes stay close to the mean."""
    half = (src >= HALF).astype(np.int64)
    deg0 = np.bincount(dst[half == 0], minlength=N_NODES)
    deg1 = np.bincount(dst[half == 1], minlength=N_NODES)
    deg = deg0 + deg1
    order = np.argsort(-deg, kind="stable")
    n_bins = N_CORES * NODE_TILES
    load = np.zeros((n_bins, 2), dtype=np.int64)
    fill = np.zeros(n_bins, dtype=np.int32)
    node_bin = np.empty(N_NODES, dtype=np.int32)
    INF = np.int64(1) << 50
    for n in order:
        d0, d1 = deg0[n], deg1[n]
        # quadratic potential: prefer the bin where the load stays most even
        cost = (load[:, 0] + d0) ** 2 + (load[:, 1] + d1) ** 2
        cost = np.where(fill < 128, cost, INF)
        b = int(np.argmin(cost))
        node_bin[n] = b
        load[b, 0] += d0
        load[b, 1] += d1
        fill[b] += 1
    node_core = (node_bin // NODE_TILES).astype(np.int32)
    node_tile = (node_bin % NODE_TILES).astype(np.int32)
    node_slot = np.empty(N_NODES, dtype=np.int32)
    fill[:] = 0
    for n in order:
        b = node_bin[n]
        node_slot[n] = fill[b]
        fill[b] += 1
    perm = -np.ones((N_CORES, SLOTS), dtype=np.int64)
    perm[node_core, node_tile * 128 + node_slot] = np.arange(N_NODES)
    return node_core, node_tile, node_slot, perm


def _build_schedule(src, dst, rel, inv):
    node_core, node_tile, node_slot, perm = _assign_nodes(src, dst)
    core = node_core[dst]
    r2 = rel + N_REL * inv
    half = (src >= HALF).astype(np.int64)
    gidx_all = np.where(half == 0, src, src - HALF)

    tile_of = node_tile[dst]
    key = (half * NODE_TILES + tile_of).astype(np.int64)
    n_groups = 2 * NODE_TILES

    counts = np.zeros((N_CORES, n_groups), dtype=np.int64)
    for c in range(N_CORES):
        counts[c] = np.bincount(key[core == c], minlength=n_groups)
    # per-half padded sizes (shared across cores); tiles may straddle
    # adjacent dst_tile groups within a half
    half_sizes = counts.reshape(N_CORES, 2, NODE_TILES).sum(axis=2).max(axis=0)
    half_pad = (half_sizes + 127) // 128 * 128
    epad = int(half_pad.sum())
    half_split = int(half_pad[0])
    hoff = np.array([0, half_split])

    gidx = np.zeros((N_CORES, epad), dtype=np.int16)
    dslot0 = np.full((N_CORES, epad), PAD_SLOT, dtype=np.float32)
    dslot1 = np.full((N_CORES, epad), PAD_SLOT, dtype=np.float32)
    # per-edge dst_tile id (for boundary-tile session structure), -1 for pads
    tileid = np.full((N_CORES, epad), -1, dtype=np.int32)
    # count matrix per core: C_T[r2, slot_global], stored chunked for DMA:
    # ct8[p, ch, col] = C_T[ch*125 + p, col]
    ct8 = np.zeros((N_CORES, 125, 8, SLOTS), dtype=ml_dtypes.bfloat16)
    for c in range(N_CORES):
        m = core == c
        k = key[m]
        o = np.argsort(k, kind="stable")
        ks = k[o]
        h_of = ks // NODE_TILES
        pos = np.arange(len(ks)) - np.searchsorted(ks, h_of * NODE_TILES)
        at = hoff[h_of] + pos
        gidx[c, at] = gidx_all[m][o].astype(np.int16)
        sl = node_slot[dst[m][o]].astype(np.float32)
        iv = inv[m][o]
        dslot0[c, at] = np.where(iv == 0, sl, PAD_SLOT)
        dslot1[c, at] = np.where(iv == 1, sl, PAD_SLOT)
        tileid[c, at] = (ks % NODE_TILES).astype(np.int32)
        colg = (node_tile[dst[m]] * 128 + node_slot[dst[m]]).astype(np.int64)
        flat = np.bincount(colg * R2 + r2[m], minlength=SLOTS * R2)
        ct = flat.reshape(SLOTS, R2).T.astype(np.float32)  # [R2, SLOTS]
        ct8[c] = ct.reshape(8, 125, SLOTS).transpose(1, 0, 2).astype(
            ml_dtypes.bfloat16)

    # static tile plan: for each 128-edge tile and each dst_tile it can
    # touch on ANY core, one (matmul-pair, compare) slot. A dst_tile's
    # session spans [first_tile, last_tile] maxed over cores.
    n_tiles = epad // 128
    touch = np.zeros((n_tiles, NODE_TILES), dtype=bool)
    for c in range(N_CORES):
        tv = tileid[c].reshape(n_tiles, 128)
        for t in range(NODE_TILES):
            touch[:, t] |= (tv == t).any(axis=1)
    # sessions must be contiguous tile ranges; fill gaps (cheap)
    for t in range(NODE_TILES):
        w = np.nonzero(touch[:, t])[0]
        for h in range(2):
            lo_t, hi_t = hoff[h] // 128, (hoff[h] + half_pad[h]) // 128
            wh = w[(w >= lo_t) & (w < hi_t)]
            if len(wh):
                touch[wh[0]:wh[-1] + 1, t] = True

    # per-core dspair entries exist per (tile, touched dst_tile): the host
    # rewrites slots relative to each touched dst_tile (255 when the edge
    # belongs to a different dst_tile)
    return dict(perm=perm, epad=epad, half_split=half_split, gidx=gidx,
                dslot0=dslot0, dslot1=dslot1, tileid=tileid, touch=touch,
                ct8=ct8)


def _pair_plan(touch, half_split, epad):
    """Static (tile, dst_tile) pair list + session open/close markers."""
    n_tiles = epad // 128
    hs_tile = half_split // 128
    pairs = []
    for tile in range(n_tiles):
        for t in np.nonzero(touch[tile])[0]:
            pairs.append((tile, int(t)))
    first = {}
    last = {}
    for tile, t in pairs:
        h = 0 if tile < hs_tile else 1
        if (h, t) not in first:
            first[(h, t)] = tile
        last[(h, t)] = tile
    return pairs, first, last


def _build_dspair(dslot0, dslot1, tileid, pairs):
    """[128, 2*n_pairs] f32: per pair, slots masked to that dst_tile."""
    n_pairs = len(pairs)
    d0 = dslot0.reshape(-1, 128)
    d1 = dslot1.reshape(-1, 128)
    tv = tileid.reshape(-1, 128)
    out = np.full((128, 2 * n_pairs), PAD_SLOT, dtype=np.float32)
    for p, (tile, t) in enumerate(pairs):
        m = tv[tile] == t
        out[:, 2 * p] = np.where(m, d0[tile], PAD_SLOT)
        out[:, 2 * p + 1] = np.where(m, d1[tile], PAD_SLOT)
    return out


def _hilo(a):
    hi = a.astype(ml_dtypes.bfloat16)
    lo = (a - hi.astype(np.float32)).astype(ml_dtypes.bfloat16)
    return np.concatenate([hi, lo], axis=-1).copy()


def _wrap16(a):
    return np.tile(a.reshape(-1, 16).T, (8, 1)).copy()


def _slot_layout(a):
    return a.reshape(-1, 128).T.copy()


# ------------------------------------------------------------ bass program

def _build_program(pairs, first, last, epad, half_split):
    import os
    from concourse import bass, bacc, tile
    mybir = bass.mybir
    f32, bf16, i16 = mybir.dt.float32, mybir.dt.bfloat16, mybir.dt.int16
    add = mybir.AluOpType.add

    nc = bacc.Bacc(None, target_bir_lowering=False)
    dp = nc.declare_dram_parameter
    vtab = dp("vtab", [N_NODES, 2 * D], bf16, isOutput=False)
    metarelT = dp("metarelT", [D, 512], f32, isOutput=False)
    wia_t = dp("wia_t", [D, D], f32, isOutput=False)
    woa_t = dp("woa_t", [D, D], f32, isOutput=False)
    wib_t = dp("wib_t", [D, D], f32, isOutput=False)
    wob_t = dp("wob_t", [D, D], f32, isOutput=False)
    ws_t = dp("ws_t", [D, D], f32, isOutput=False)
    wm_t = dp("wm_t", [D, D], f32, isOutput=False)
    bias_i = dp("bias_i", [128, D], f32, isOutput=False)
    bias_o = dp("bias_o", [128, D], f32, isOutput=False)
    bias_s = dp("bias_s", [128, D], f32, isOutput=False)
    bias_m = dp("bias_m", [128, D], f32, isOutput=False)
    n_pairs = len(pairs)
    iota2 = dp("iota2", [128, 256], f32, isOutput=False)
    gidx = dp("gidx", [128, epad // 16], i16, isOutput=False)
    dspair = dp("dspair", [128, 2 * n_pairs], f32, isOutput=False)
    ct8 = dp("ct8", [125, 8 * SLOTS], bf16, isOutput=False)
    relembT = dp("relembT", [D, SLOTS], f32, isOutput=False)
    out_rel = dp("out_rel", [SLOTS, D], f32, isOutput=True)
    out_meta = dp("out_meta", [N_REL, D], f32, isOutput=True)

    n_tiles = epad // 128

    with tile.TileContext(nc) as tc:
        with tc.tile_pool(name="const", bufs=1) as cpool, \
             tc.tile_pool(name="acc", bufs=1) as apool, \
             tc.tile_pool(name="buf", bufs=4) as bpool, \
             tc.tile_pool(name="oh", bufs=6) as ohpool, \
             tc.tile_pool(name="work", bufs=3) as wpool, \
             tc.tile_pool(name="stage", bufs=3) as spool, \
             tc.tile_pool(name="ct", bufs=3) as ctpool, \
             tc.tile_pool(name="ps", bufs=4, space="PSUM") as pspool, \
             tc.tile_pool(name="ps2", bufs=2, space="PSUM") as ps2pool, \
             tc.tile_pool(name="psx", bufs=2, space="PSUM") as psxpool:

            # ---- constants to SBUF
            def ld(nm, shape, dt_, src):
                t_ = cpool.tile(shape, dt_, tag=nm, name=nm)
                nc.sync.dma_start(out=t_[:], in_=src[:])
                return t_

            metarelT_sb = ld("metarelT_sb", [D, 512], f32, metarelT)
            wia_sb = ld("wia_sb", [D, D], f32, wia_t)
            woa_sb = ld("woa_sb", [D, D], f32, woa_t)
            wib_sb = ld("wib_sb", [D, D], f32, wib_t)
            wob_sb = ld("wob_sb", [D, D], f32, wob_t)
            ws_sb = ld("ws_sb", [D, D], f32, ws_t)
            wm_sb = ld("wm_sb", [D, D], f32, wm_t)
            bias_i_sb = ld("bias_i_sb", [128, D], f32, bias_i)
            bias_o_sb = ld("bias_o_sb", [128, D], f32, bias_o)
            bias_s_sb = ld("bias_s_sb", [128, D], f32, bias_s)
            bias_m_sb = ld("bias_m_sb", [128, D], f32, bias_m)
            iota2_sb = ld("iota2_sb", [128, 256], f32, iota2)
            gidx_sb = cpool.tile([128, epad // 16], i16, tag="gidx_sb",
                                 name="gidx_sb")
            c0 = min(BATCH // 16, epad // 16)
            nc.sync.dma_start(out=gidx_sb[:, :c0], in_=gidx[:, :c0])
            nc.sync.dma_start(out=gidx_sb[:, c0:], in_=gidx[:, c0:])
            dspair_sb = ld("dspair_sb", [128, 2 * n_pairs], f32, dspair)

            # ---- phase 0: P tables -> SBUF rhs [125, 8, 257] = [hi|lo|1]
            p_rhs = cpool.tile([125, 8, 2 * D + 1], bf16, tag="p_rhs",
                               name="p_rhs")
            nc.vector.memset(p_rhs[:], 1.0)  # ones column (rest overwritten)
            for ti, (w_sb, b_sb) in enumerate(((wia_sb, bias_i_sb),
                                               (woa_sb, bias_o_sb))):
                for ch4 in range(4):
                    rows = 125
                    col0 = ch4 * 125
                    ch = ti * 4 + ch4
                    psp = psxpool.tile([128, 2 * D + 1], f32, tag="psx",
                                       name="psp")
                    nc.tensor.matmul(psp[:rows, 0:D],
                                     metarelT_sb[:, col0:col0 + rows],
                                     w_sb[:], start=True, stop=True)
                    pf = wpool.tile([128, D], f32, tag="pf", name="pf")
                    nc.vector.tensor_tensor(pf[:rows], psp[:rows, 0:D],
                                            b_sb[:rows], add)
                    nc.vector.tensor_copy(p_rhs[:, ch, 0:D], pf[:rows])
                    phi32 = wpool.tile([128, D], f32, tag="phi32",
                                       name="phi32")
                    nc.vector.tensor_copy(phi32[:rows], p_rhs[:, ch, 0:D])
                    nc.vector.tensor_tensor(p_rhs[:, ch, D:2 * D], pf[:rows],
                                            phi32[:rows],
                                            mybir.AluOpType.subtract)

            # ---- accumulators
            sp_acc = [apool.tile([128, D], f32, tag=f"sp{t}", name=f"sp{t}")
                      for t in range(NODE_TILES)]
            s0_acc = [apool.tile([128, D], f32, tag=f"s0{t}", name=f"s0{t}")
                      for t in range(NODE_TILES)]  # [fin, nodes] layout
            s1_acc = [apool.tile([128, D], f32, tag=f"s1{t}", name=f"s1{t}")
                      for t in range(NODE_TILES)]  # [fin, nodes] layout
            cnt_acc = [apool.tile([128, 1], f32, tag=f"cnt{t}",
                                  name=f"cnt{t}")
                       for t in range(NODE_TILES)]

            # ---- C phase (emitted lazily per node tile):
            # SP = C @ (P_hi + P_lo), cnt = C @ 1
            ct8_3d = ct8[:, :].rearrange("p (c s) -> p c s", c=8)

            def emit_cphase(t):
                ctile = ctpool.tile([125, 8, 128], bf16, tag="ctile",
                                    name="ctile")
                nc.sync.dma_start(
                    out=ctile[:],
                    in_=ct8_3d[:, :, t * 128:(t + 1) * 128])
                psC = psxpool.tile([128, 2 * D + 1], f32, tag="psx",
                                   name="psC")
                for ch in range(8):
                    nc.tensor.matmul(psC[:], ctile[:, ch, :],
                                     p_rhs[:, ch, :],
                                     start=(ch == 0), stop=(ch == 7))
                nc.vector.tensor_copy(sp_acc[t][:], psC[:, 0:D])
                nc.vector.tensor_tensor(sp_acc[t][:], sp_acc[t][:],
                                        psC[:, D:2 * D], add)
                nc.vector.tensor_scalar(cnt_acc[t][:], psC[:, 2 * D:2 * D + 1],
                                        1.0, None, op0=mybir.AluOpType.max)
                nc.vector.reciprocal(cnt_acc[t][:], cnt_acc[t][:])

            # ---- edge phase
            CH = BATCH // 128
            batches = []
            hs_tile = half_split // 128
            for lo, hi_ in ((0, hs_tile), (hs_tile, n_tiles)):
                s = lo
                while s < hi_:
                    n = min(CH, hi_ - s)
                    batches.append((s, n))
                    s += n
            tile2batch = {}
            for bi, (s, n) in enumerate(batches):
                for t_ in range(s, s + n):
                    tile2batch[t_] = (bi, t_ - s)

            bufs = {}
            # ---- meta_new (emitted early; independent of edges)
            for ch4 in range(4):
                rows = 125
                col0 = ch4 * 125
                psm = psxpool.tile([128, 2 * D + 1], f32, tag="psx",
                                   name="psm")
                nc.tensor.matmul(psm[:rows, 0:D],
                                 metarelT_sb[:, col0:col0 + rows],
                                 wm_sb[:], start=True, stop=True)
                om = spool.tile([128, D], f32, tag="om", name="om")
                nc.vector.tensor_tensor(om[:rows], psm[:rows, 0:D],
                                        bias_m_sb[:rows], add)
                nc.sync.dma_start(out=out_meta[col0:col0 + rows, :],
                                  in_=om[:rows])

            def emit_post(t):
                ps2 = ps2pool.tile([128, D], f32, tag="ps2", name="ps2")
                nc.tensor.matmul(ps2[:], s0_acc[t][:], wib_sb[:],
                                 start=True, stop=False)
                nc.tensor.matmul(ps2[:], s1_acc[t][:], wob_sb[:],
                                 start=False, stop=True)
                ret = spool.tile([128, D], f32, tag="ret", name="ret")
                nc.sync.dma_start(out=ret[:],
                                  in_=relembT[:, t * 128:(t + 1) * 128])
                ps3 = ps2pool.tile([128, D], f32, tag="ps2", name="ps3")
                nc.tensor.matmul(ps3[:], ret[:], ws_sb[:],
                                 start=True, stop=True)
                o1 = spool.tile([128, D], f32, tag="o1", name="o1")
                nc.vector.tensor_tensor(o1[:], ps2[:], sp_acc[t][:], add)
                nc.vector.tensor_tensor(o1[:], o1[:],
                                        cnt_acc[t][:].broadcast_to([128, D]),
                                        mybir.AluOpType.mult)
                nc.vector.tensor_tensor(o1[:], o1[:], ps3[:], add)
                nc.vector.tensor_tensor(o1[:], o1[:], bias_s_sb[:], add)
                nc.sync.dma_start(out=out_rel[t * 128:(t + 1) * 128, :],
                                  in_=o1[:])

            ps_live = {}
            for p_idx, (t_idx, t) in enumerate(pairs):
                h = 0 if t_idx < hs_tile else 1
                is_first = first[(h, t)] == t_idx
                is_last = last[(h, t)] == t_idx
                if h == 1 and is_first:
                    emit_cphase(t)
                bi, c = tile2batch[t_idx]
                if c == 0 and bi not in bufs:
                    s, n = batches[bi]
                    buf = bpool.tile([128, CH, 2 * D], bf16, tag="gbuf",
                                     name="gbuf")
                    e0, ne = s * 128, n * 128
                    vbase = 0 if t_idx < hs_tile else HALF
                    nc.gpsimd.dma_gather(
                        buf[:, :n], vtab[vbase:vbase + HALF, :],
                        gidx_sb[:, e0 // 16:(e0 + ne) // 16], ne, ne,
                        2 * D, single_packet=False)
                    bufs[bi] = buf
                buf = bufs[bi]

                oh = ohpool.tile([128, 2, 128], bf16, tag="oh", name="oh")
                nc.vector.tensor_tensor(
                    oh[:], iota2_sb[:].rearrange("p (b j) -> p b j", b=2),
                    dspair_sb[:, 2 * p_idx:2 * p_idx + 2]
                    .broadcast_to([128, 2, 128]),
                    mybir.AluOpType.is_equal)
                if is_first:
                    ps_live[t] = pspool.tile([128, 256], f32, tag="ps",
                                             name="ps")
                ps = ps_live[t]
                nc.tensor.matmul(ps[:], buf[:, c, 0:D], oh[:],
                                 start=is_first, stop=False)
                nc.tensor.matmul(ps[:], buf[:, c, D:2 * D], oh[:],
                                 start=False, stop=is_last)

                if is_last:
                    del ps_live[t]
                    if h == 0:
                        nc.vector.tensor_copy(s0_acc[t][:], ps[:, 0:D])
                        nc.vector.tensor_copy(s1_acc[t][:], ps[:, D:2 * D])
                    else:
                        nc.vector.tensor_tensor(s0_acc[t][:], s0_acc[t][:],
                                                ps[:, 0:D], add)
                        nc.vector.tensor_tensor(s1_acc[t][:], s1_acc[t][:],
                                                ps[:, D:2 * D], add)
                        emit_post(t)


    nc.finalize()
    return nc


_CACHE = {}


def kernel(rel_emb, metarel_emb, src, dst, rel, inv,
           W_I_w, W_I_b, W_O_w, W_O_b, W_S_w, W_S_b, W_M_w, W_M_b):
    from concourse.bass_utils import run_bass_kernel_spmd

    rel_emb = np.asarray(rel_emb, dtype=np.float32)
    metarel_emb = np.asarray(metarel_emb, dtype=np.float32)
    src = np.asarray(src).astype(np.int64)
    dst = np.asarray(dst).astype(np.int64)
    rel = np.asarray(rel).astype(np.int64)
    inv = np.asarray(inv).astype(np.int64)
    W_I_w = np.asarray(W_I_w, dtype=np.float32)
    W_O_w = np.asarray(W_O_w, dtype=np.float32)
    W_S_w = np.asarray(W_S_w, dtype=np.float32)
    W_M_w = np.asarray(W_M_w, dtype=np.float32)

    sched = _build_schedule(src, dst, rel, inv)
    pairs, first, last = _pair_plan(sched['touch'], sched['half_split'],
                                    sched['epad'])
    key = (int(sched['epad']), int(sched['half_split']),
           sched['touch'].tobytes())
    if key not in _CACHE:
        _CACHE[key] = _build_program(pairs, first, last, sched['epad'],
                                     sched['half_split'])
    nc = _CACHE[key]

    vtab = _hilo(rel_emb)
    metarelT = np.zeros((D, 512), np.float32)
    metarelT[:, :N_REL] = metarel_emb.T
    bcast = lambda b: np.tile(np.asarray(b, np.float32)[None, :],
                              (128, 1)).copy()

    shared = {
        "vtab": vtab,
        "metarelT": metarelT,
        "wia_t": W_I_w[:, :D].T.copy(), "woa_t": W_O_w[:, :D].T.copy(),
        "wib_t": W_I_w[:, D:].T.copy(), "wob_t": W_O_w[:, D:].T.copy(),
        "ws_t": W_S_w.T.copy(), "wm_t": W_M_w.T.copy(),
        "bias_i": bcast(W_I_b), "bias_o": bcast(W_O_b),
        "bias_s": bcast(W_S_b), "bias_m": bcast(W_M_b),
        "iota2": np.tile(np.arange(128, dtype=np.float32), (128, 2)).copy(),
    }
    perm = sched['perm']
    in_maps = []
    for c in range(N_CORES):
        rows = np.where(perm[c] >= 0, perm[c], 0)
        relembT = rel_emb[rows].T.copy()
        dspair = _build_dspair(sched['dslot0'][c], sched['dslot1'][c],
                               sched['tileid'][c], pairs)
        in_maps.append(dict(shared,
                            gidx=_wrap16(sched['gidx'][c]),
                            dspair=dspair,
                            ct8=sched['ct8'][c].reshape(125, 8 * SLOTS).copy(),
                            relembT=relembT))

    try:
        res = run_bass_kernel_spmd(nc, in_maps, list(range(N_CORES)))
    except Exception:
        # transient NRT device errors (e.g. a previous crashed process left
        # a core wedged) usually clear on retry
        res = run_bass_kernel_spmd(nc, in_maps, list(range(N_CORES)))
    globals()['_last_results'] = res

    rel_new = np.zeros((N_NODES, D), np.float32)
    for c in range(N_CORES):
        valid = perm[c] >= 0
        rel_new[perm[c][valid]] = res.results[c]["out_rel"][valid]
    meta_new = res.results[0]["out_meta"]
    return rel_new, meta_new


# revision 22
# speedup vs baseline: 1.0212x; 1.0029x over previous
"""Trainium2 Bass kernel for nn_Ext_relLayer (GNN message passing).

Strategy (edge-parallel, sharded by destination-node ownership):
  * The per-edge masked linear is decomposed by linearity:
      msg[e] = P_{inv}[rel[e]] + rel_emb[src[e]] @ W_{inv,B}.T
    where P_i = metarel_emb @ W_{i,A}.T + b_i  (A = first D cols, B = last D).
    The W_B matmul is pushed past the segment-sum, so the edge phase is pure
    gather + masked segment-sum:
      agg[n] = SP[n] + S0[n] @ W_IB.T + S1[n] @ W_OB.T
      SP[n] = sum_{e->n} P_{r2[e]},  Si[n] = sum_{e->n, inv=i} rel_emb[src[e]]
  * SP and the in-degree counts are computed without any per-edge work: the
    host bincounts (node, rel2) pairs into a count matrix C (pure index
    preprocessing), and the device computes SP = C @ P and cnt = C @ 1 as
    dense PE matmuls against the on-chip P table.
  * 50000 nodes are assigned to 8 cores x 49 tiles x 128 slots, balancing
    per-tile edge counts per src-half. Each edge lives on the core owning
    its dst. Per core, edges are ordered by (src_half, dst_tile); groups are
    padded to a multiple of 128 (sizes maxed over cores -> one static SPMD
    program).
  * Edge phase per 128-edge tile: dma_gather 512B rows v = rel_emb[src]
    stored as [bf16(x) | bf16(x - hi)] pairs; two masked one-hots
    (dst_slot_inv0/1 == iota) scatter-add v into S0/S1 PSUM columns via
    TensorE (hi+lo matmuls accumulate -> exact to ~2^-17).
  * Post phase per node tile: transpose S0/S1 on TensorE, apply W_IB/W_OB,
    add SP, scale by 1/max(cnt,1), add rel_emb @ W_S.T + b_S. meta_new is a
    tiny dense matmul done on every core (host takes core 0's copy).
"""
import sys
import numpy as np

sys.path.insert(0, '/opt/trn_rl_repo')

import ml_dtypes  # noqa: E402

N_NODES, N_EDGES, N_REL, D = 50000, 640000, 500, 128
R2 = 2 * N_REL               # (rel, inv) alphabet
N_CORES = 8
NODE_TILES = 49              # node tiles per core
SLOTS = NODE_TILES * 128     # node slots per core
HALF = 25000                 # src table split for int16 gather indices
PAD_SLOT = 255.0
BATCH = 4096                 # edges per dma_gather call


# ---------------------------------------------------------------- host prep

def _assign_nodes(src, dst):
    """Assign nodes to (core, tile, slot) balancing per-(bin, src_half) edge
    counts so the cross-core maxed group sizes stay close to the mean."""
    half = (src >= HALF).astype(np.int64)
    deg0 = np.bincount(dst[half == 0], minlength=N_NODES)
    deg1 = np.bincount(dst[half == 1], minlength=N_NODES)
    deg = deg0 + deg1
    order = np.argsort(-deg, kind="stable")
    n_bins = N_CORES * NODE_TILES
    load = np.zeros((n_bins, 2), dtype=np.int64)
    fill = np.zeros(n_bins, dtype=np.int32)
    node_bin = np.empty(N_NODES, dtype=np.int32)
    INF = np.int64(1) << 50
    for n in order:
        d0, d1 = deg0[n], deg1[n]
        # quadratic potential: prefer the bin where the load stays most even
        cost = (load[:, 0] + d0) ** 2 + (load[:, 1] + d1) ** 2
        cost = np.where(fill < 128, cost, INF)
        b = int(np.argmin(cost))
        node_bin[n] = b
        load[b, 0] += d0
        load[b, 1] += d1
        fill[b] += 1
    node_core = (node_bin // NODE_TILES).astype(np.int32)
    node_tile = (node_bin % NODE_TILES).astype(np.int32)
    node_slot = np.empty(N_NODES, dtype=np.int32)
    fill[:] = 0
    for n in order:
        b = node_bin[n]
        node_slot[n] = fill[b]
        fill[b] += 1
    perm = -np.ones((N_CORES, SLOTS), dtype=np.int64)
    perm[node_core, node_tile * 128 + node_slot] = np.arange(N_NODES)
    return node_core, node_tile, node_slot, perm


def _build_schedule(src, dst, rel, inv):
    node_core, node_tile, node_slot, perm = _assign_nodes(src, dst)
    core = node_core[dst]
    r2 = rel + N_REL * inv
    half = (src >= HALF).astype(np.int64)
    gidx_all = np.where(half == 0, src, src - HALF)

    tile_of = node_tile[dst]
    key = (half * NODE_TILES + tile_of).astype(np.int64)
    n_groups = 2 * NODE_TILES

    counts = np.zeros((N_CORES, n_groups), dtype=np.int64)
    for c in range(N_CORES):
        counts[c] = np.bincount(key[core == c], minlength=n_groups)
    # per-half padded sizes (shared across cores); tiles may straddle
    # adjacent dst_tile groups within a half
    half_sizes = counts.reshape(N_CORES, 2, NODE_TILES).sum(axis=2).max(axis=0)
    half_pad = (half_sizes + 127) // 128 * 128
    epad = int(half_pad.sum())
    half_split = int(half_pad[0])
    hoff = np.array([0, half_split])

    gidx = np.zeros((N_CORES, epad), dtype=np.int16)
    dslot0 = np.full((N_CORES, epad), PAD_SLOT, dtype=np.float32)
    dslot1 = np.full((N_CORES, epad), PAD_SLOT, dtype=np.float32)
    # per-edge dst_tile id (for boundary-tile session structure), -1 for pads
    tileid = np.full((N_CORES, epad), -1, dtype=np.int32)
    # count matrix per core: C_T[r2, slot_global], stored chunked for DMA:
    # ct8[p, ch, col] = C_T[ch*125 + p, col]
    ct8 = np.zeros((N_CORES, 125, 8, SLOTS), dtype=ml_dtypes.bfloat16)
    for c in range(N_CORES):
        m = core == c
        k = key[m]
        o = np.argsort(k, kind="stable")
        ks = k[o]
        h_of = ks // NODE_TILES
        pos = np.arange(len(ks)) - np.searchsorted(ks, h_of * NODE_TILES)
        at = hoff[h_of] + pos
        gidx[c, at] = gidx_all[m][o].astype(np.int16)
        sl = node_slot[dst[m][o]].astype(np.float32)
        iv = inv[m][o]
        dslot0[c, at] = np.where(iv == 0, sl, PAD_SLOT)
        dslot1[c, at] = np.where(iv == 1, sl, PAD_SLOT)
        tileid[c, at] = (ks % NODE_TILES).astype(np.int32)
        colg = (node_tile[dst[m]] * 128 + node_slot[dst[m]]).astype(np.int64)
        flat = np.bincount(colg * R2 + r2[m], minlength=SLOTS * R2)
        ct = flat.reshape(SLOTS, R2).T.astype(np.float32)  # [R2, SLOTS]
        ct8[c] = ct.reshape(8, 125, SLOTS).transpose(1, 0, 2).astype(
            ml_dtypes.bfloat16)

    # static tile plan: for each 128-edge tile and each dst_tile it can
    # touch on ANY core, one (matmul-pair, compare) slot. A dst_tile's
    # session spans [first_tile, last_tile] maxed over cores.
    n_tiles = epad // 128
    touch = np.zeros((n_tiles, NODE_TILES), dtype=bool)
    for c in range(N_CORES):
        tv = tileid[c].reshape(n_tiles, 128)
        for t in range(NODE_TILES):
            touch[:, t] |= (tv == t).any(axis=1)
    # sessions must be contiguous tile ranges; fill gaps (cheap)
    for t in range(NODE_TILES):
        w = np.nonzero(touch[:, t])[0]
        for h in range(2):
            lo_t, hi_t = hoff[h] // 128, (hoff[h] + half_pad[h]) // 128
            wh = w[(w >= lo_t) & (w < hi_t)]
            if len(wh):
                touch[wh[0]:wh[-1] + 1, t] = True

    # per-core dspair entries exist per (tile, touched dst_tile): the host
    # rewrites slots relative to each touched dst_tile (255 when the edge
    # belongs to a different dst_tile)
    return dict(perm=perm, epad=epad, half_split=half_split, gidx=gidx,
                dslot0=dslot0, dslot1=dslot1, tileid=tileid, touch=touch,
                ct8=ct8)


def _pair_plan(touch, half_split, epad):
    """Static (tile, dst_tile) pair list + session open/close markers."""
    n_tiles = epad // 128
    hs_tile = half_split // 128
    pairs = []
    for tile in range(n_tiles):
        for t in np.nonzero(touch[tile])[0]:
            pairs.append((tile, int(t)))
    first = {}
    last = {}
    for tile, t in pairs:
        h = 0 if tile < hs_tile else 1
        if (h, t) not in first:
            first[(h, t)] = tile
        last[(h, t)] = tile
    return pairs, first, last


def _build_dspair(dslot0, dslot1, tileid, pairs):
    """[128, 2*n_pairs] f32: per pair, slots masked to that dst_tile."""
    n_pairs = len(pairs)
    d0 = dslot0.reshape(-1, 128)
    d1 = dslot1.reshape(-1, 128)
    tv = tileid.reshape(-1, 128)
    out = np.full((128, 2 * n_pairs), PAD_SLOT, dtype=np.float32)
    for p, (tile, t) in enumerate(pairs):
        m = tv[tile] == t
        out[:, 2 * p] = np.where(m, d0[tile], PAD_SLOT)
        out[:, 2 * p + 1] = np.where(m, d1[tile], PAD_SLOT)
    return out


def _hilo(a):
    hi = a.astype(ml_dtypes.bfloat16)
    lo = (a - hi.astype(np.float32)).astype(ml_dtypes.bfloat16)
    return np.concatenate([hi, lo], axis=-1).copy()


def _wrap16(a):
    return np.tile(a.reshape(-1, 16).T, (8, 1)).copy()


def _slot_layout(a):
    return a.reshape(-1, 128).T.copy()


# ------------------------------------------------------------ bass program

def _build_program(pairs, first, last, epad, half_split):
    import os
    from concourse import bass, bacc, tile
    mybir = bass.mybir
    f32, bf16, i16 = mybir.dt.float32, mybir.dt.bfloat16, mybir.dt.int16
    add = mybir.AluOpType.add

    nc = bacc.Bacc(None, target_bir_lowering=False)
    dp = nc.declare_dram_parameter
    vtab = dp("vtab", [N_NODES, 2 * D], bf16, isOutput=False)
    metarelT = dp("metarelT", [D, 512], f32, isOutput=False)
    wia_t = dp("wia_t", [D, D], f32, isOutput=False)
    woa_t = dp("woa_t", [D, D], f32, isOutput=False)
    wib_t = dp("wib_t", [D, D], f32, isOutput=False)
    wob_t = dp("wob_t", [D, D], f32, isOutput=False)
    ws_t = dp("ws_t", [D, D], f32, isOutput=False)
    wm_t = dp("wm_t", [D, D], f32, isOutput=False)
    bias_i = dp("bias_i", [128, D], f32, isOutput=False)
    bias_o = dp("bias_o", [128, D], f32, isOutput=False)
    bias_s = dp("bias_s", [128, D], f32, isOutput=False)
    bias_m = dp("bias_m", [128, D], f32, isOutput=False)
    n_pairs = len(pairs)
    iota2 = dp("iota2", [128, 256], f32, isOutput=False)
    gidx = dp("gidx", [128, epad // 16], i16, isOutput=False)
    dspair = dp("dspair", [128, 2 * n_pairs], f32, isOutput=False)
    ct8 = dp("ct8", [125, 8 * SLOTS], bf16, isOutput=False)
    relembT = dp("relembT", [D, SLOTS], f32, isOutput=False)
    out_rel = dp("out_rel", [SLOTS, D], f32, isOutput=True)
    out_meta = dp("out_meta", [N_REL, D], f32, isOutput=True)

    n_tiles = epad // 128

    with tile.TileContext(nc) as tc:
        with tc.tile_pool(name="const", bufs=1) as cpool, \
             tc.tile_pool(name="acc", bufs=1) as apool, \
             tc.tile_pool(name="buf", bufs=4) as bpool, \
             tc.tile_pool(name="oh", bufs=6) as ohpool, \
             tc.tile_pool(name="work", bufs=3) as wpool, \
             tc.tile_pool(name="stage", bufs=3) as spool, \
             tc.tile_pool(name="ct", bufs=3) as ctpool, \
             tc.tile_pool(name="ps", bufs=4, space="PSUM") as pspool, \
             tc.tile_pool(name="ps2", bufs=2, space="PSUM") as ps2pool, \
             tc.tile_pool(name="psx", bufs=2, space="PSUM") as psxpool:

            # ---- constants to SBUF
            def ld(nm, shape, dt_, src):
                t_ = cpool.tile(shape, dt_, tag=nm, name=nm)
                nc.sync.dma_start(out=t_[:], in_=src[:])
                return t_

            gidx_sb = cpool.tile([128, epad // 16], i16, tag="gidx_sb",
                                 name="gidx_sb")
            c0 = min(BATCH // 16, epad // 16)
            nc.sync.dma_start(out=gidx_sb[:, :c0], in_=gidx[:, :c0])
            metarelT_sb = ld("metarelT_sb", [D, 512], f32, metarelT)
            wia_sb = ld("wia_sb", [D, D], f32, wia_t)
            woa_sb = ld("woa_sb", [D, D], f32, woa_t)
            wib_sb = ld("wib_sb", [D, D], f32, wib_t)
            wob_sb = ld("wob_sb", [D, D], f32, wob_t)
            ws_sb = ld("ws_sb", [D, D], f32, ws_t)
            wm_sb = ld("wm_sb", [D, D], f32, wm_t)
            bias_i_sb = ld("bias_i_sb", [128, D], f32, bias_i)
            bias_o_sb = ld("bias_o_sb", [128, D], f32, bias_o)
            bias_s_sb = ld("bias_s_sb", [128, D], f32, bias_s)
            bias_m_sb = ld("bias_m_sb", [128, D], f32, bias_m)
            iota2_sb = ld("iota2_sb", [128, 256], f32, iota2)
            nc.sync.dma_start(out=gidx_sb[:, c0:], in_=gidx[:, c0:])
            dspair_sb = ld("dspair_sb", [128, 2 * n_pairs], f32, dspair)

            # ---- phase 0: P tables -> SBUF rhs [125, 8, 257] = [hi|lo|1]
            p_rhs = cpool.tile([125, 8, 2 * D + 1], bf16, tag="p_rhs",
                               name="p_rhs")
            nc.vector.memset(p_rhs[:], 1.0)  # ones column (rest overwritten)
            for ti, (w_sb, b_sb) in enumerate(((wia_sb, bias_i_sb),
                                               (woa_sb, bias_o_sb))):
                for ch4 in range(4):
                    rows = 125
                    col0 = ch4 * 125
                    ch = ti * 4 + ch4
                    psp = psxpool.tile([128, 2 * D + 1], f32, tag="psx",
                                       name="psp")
                    nc.tensor.matmul(psp[:rows, 0:D],
                                     metarelT_sb[:, col0:col0 + rows],
                                     w_sb[:], start=True, stop=True)
                    pf = wpool.tile([128, D], f32, tag="pf", name="pf")
                    nc.vector.tensor_tensor(pf[:rows], psp[:rows, 0:D],
                                            b_sb[:rows], add)
                    nc.vector.tensor_copy(p_rhs[:, ch, 0:D], pf[:rows])
                    phi32 = wpool.tile([128, D], f32, tag="phi32",
                                       name="phi32")
                    nc.vector.tensor_copy(phi32[:rows], p_rhs[:, ch, 0:D])
                    nc.vector.tensor_tensor(p_rhs[:, ch, D:2 * D], pf[:rows],
                                            phi32[:rows],
                                            mybir.AluOpType.subtract)

            # ---- accumulators
            sp_acc = [apool.tile([128, D], f32, tag=f"sp{t}", name=f"sp{t}")
                      for t in range(NODE_TILES)]
            s0_acc = [apool.tile([128, D], f32, tag=f"s0{t}", name=f"s0{t}")
                      for t in range(NODE_TILES)]  # [fin, nodes] layout
            s1_acc = [apool.tile([128, D], f32, tag=f"s1{t}", name=f"s1{t}")
                      for t in range(NODE_TILES)]  # [fin, nodes] layout
            cnt_acc = [apool.tile([128, 1], f32, tag=f"cnt{t}",
                                  name=f"cnt{t}")
                       for t in range(NODE_TILES)]

            # ---- C phase (emitted lazily per node tile):
            # SP = C @ (P_hi + P_lo), cnt = C @ 1
            ct8_3d = ct8[:, :].rearrange("p (c s) -> p c s", c=8)

            def emit_cphase(t):
                ctile = ctpool.tile([125, 8, 128], bf16, tag="ctile",
                                    name="ctile")
                nc.sync.dma_start(
                    out=ctile[:],
                    in_=ct8_3d[:, :, t * 128:(t + 1) * 128])
                psC = psxpool.tile([128, 2 * D + 1], f32, tag="psx",
                                   name="psC")
                for ch in range(8):
                    nc.tensor.matmul(psC[:], ctile[:, ch, :],
                                     p_rhs[:, ch, :],
                                     start=(ch == 0), stop=(ch == 7))
                nc.vector.tensor_copy(sp_acc[t][:], psC[:, 0:D])
                nc.vector.tensor_tensor(sp_acc[t][:], sp_acc[t][:],
                                        psC[:, D:2 * D], add)
                nc.vector.tensor_scalar(cnt_acc[t][:], psC[:, 2 * D:2 * D + 1],
                                        1.0, None, op0=mybir.AluOpType.max)
                nc.vector.reciprocal(cnt_acc[t][:], cnt_acc[t][:])

            # ---- edge phase
            CH = BATCH // 128
            batches = []
            hs_tile = half_split // 128
            for lo, hi_ in ((0, hs_tile), (hs_tile, n_tiles)):
                s = lo
                while s < hi_:
                    n = min(CH, hi_ - s)
                    batches.append((s, n))
                    s += n
            # make the global final gather small so its consumption overlaps
            # earlier gathers instead of running exposed in the tail
            ls, ln = batches[-1]
            if ln > 8:
                batches[-1] = (ls, ln - 8)
                batches.append((ls + ln - 8, 8))
            tile2batch = {}
            for bi, (s, n) in enumerate(batches):
                for t_ in range(s, s + n):
                    tile2batch[t_] = (bi, t_ - s)

            bufs = {}
            # ---- meta_new (emitted early; independent of edges)
            for ch4 in range(4):
                rows = 125
                col0 = ch4 * 125
                psm = psxpool.tile([128, 2 * D + 1], f32, tag="psx",
                                   name="psm")
                nc.tensor.matmul(psm[:rows, 0:D],
                                 metarelT_sb[:, col0:col0 + rows],
                                 wm_sb[:], start=True, stop=True)
                om = spool.tile([128, D], f32, tag="om", name="om")
                nc.vector.tensor_tensor(om[:rows], psm[:rows, 0:D],
                                        bias_m_sb[:rows], add)
                nc.sync.dma_start(out=out_meta[col0:col0 + rows, :],
                                  in_=om[:rows])

            def emit_post(t):
                ps2 = ps2pool.tile([128, D], f32, tag="ps2", name="ps2")
                nc.tensor.matmul(ps2[:], s0_acc[t][:], wib_sb[:],
                                 start=True, stop=False)
                nc.tensor.matmul(ps2[:], s1_acc[t][:], wob_sb[:],
                                 start=False, stop=True)
                ret = spool.tile([128, D], f32, tag="ret", name="ret")
                nc.sync.dma_start(out=ret[:],
                                  in_=relembT[:, t * 128:(t + 1) * 128])
                ps3 = ps2pool.tile([128, D], f32, tag="ps2", name="ps3")
                nc.tensor.matmul(ps3[:], ret[:], ws_sb[:],
                                 start=True, stop=True)
                o1 = spool.tile([128, D], f32, tag="o1", name="o1")
                nc.vector.tensor_tensor(o1[:], ps2[:], sp_acc[t][:], add)
                nc.vector.tensor_tensor(o1[:], o1[:],
                                        cnt_acc[t][:].broadcast_to([128, D]),
                                        mybir.AluOpType.mult)
                nc.vector.tensor_tensor(o1[:], o1[:], ps3[:], add)
                nc.vector.tensor_tensor(o1[:], o1[:], bias_s_sb[:], add)
                nc.sync.dma_start(out=out_rel[t * 128:(t + 1) * 128, :],
                                  in_=o1[:])

            ps_live = {}
            for p_idx, (t_idx, t) in enumerate(pairs):
                h = 0 if t_idx < hs_tile else 1
                is_first = first[(h, t)] == t_idx
                is_last = last[(h, t)] == t_idx
                if h == 1 and is_first:
                    emit_cphase(t)
                bi, c = tile2batch[t_idx]
                if c == 0 and bi not in bufs:
                    s, n = batches[bi]
                    buf = bpool.tile([128, CH, 2 * D], bf16, tag="gbuf",
                                     name="gbuf")
                    e0, ne = s * 128, n * 128
                    vbase = 0 if t_idx < hs_tile else HALF
                    nc.gpsimd.dma_gather(
                        buf[:, :n], vtab[vbase:vbase + HALF, :],
                        gidx_sb[:, e0 // 16:(e0 + ne) // 16], ne, ne,
                        2 * D, single_packet=False)
                    bufs[bi] = buf
                buf = bufs[bi]

                oh = ohpool.tile([128, 2, 128], bf16, tag="oh", name="oh")
                nc.vector.tensor_tensor(
                    oh[:], iota2_sb[:].rearrange("p (b j) -> p b j", b=2),
                    dspair_sb[:, 2 * p_idx:2 * p_idx + 2]
                    .broadcast_to([128, 2, 128]),
                    mybir.AluOpType.is_equal)
                if is_first:
                    ps_live[t] = pspool.tile([128, 256], f32, tag="ps",
                                             name="ps")
                ps = ps_live[t]
                nc.tensor.matmul(ps[:], buf[:, c, 0:D], oh[:],
                                 start=is_first, stop=False)
                nc.tensor.matmul(ps[:], buf[:, c, D:2 * D], oh[:],
                                 start=False, stop=is_last)

                if is_last:
                    del ps_live[t]
                    if h == 0:
                        nc.vector.tensor_copy(s0_acc[t][:], ps[:, 0:D])
                        nc.vector.tensor_copy(s1_acc[t][:], ps[:, D:2 * D])
                    else:
                        nc.vector.tensor_tensor(s0_acc[t][:], s0_acc[t][:],
                                                ps[:, 0:D], add)
                        nc.vector.tensor_tensor(s1_acc[t][:], s1_acc[t][:],
                                                ps[:, D:2 * D], add)
                        emit_post(t)


    nc.finalize()
    return nc


_CACHE = {}


def kernel(rel_emb, metarel_emb, src, dst, rel, inv,
           W_I_w, W_I_b, W_O_w, W_O_b, W_S_w, W_S_b, W_M_w, W_M_b):
    from concourse.bass_utils import run_bass_kernel_spmd

    rel_emb = np.asarray(rel_emb, dtype=np.float32)
    metarel_emb = np.asarray(metarel_emb, dtype=np.float32)
    src = np.asarray(src).astype(np.int64)
    dst = np.asarray(dst).astype(np.int64)
    rel = np.asarray(rel).astype(np.int64)
    inv = np.asarray(inv).astype(np.int64)
    W_I_w = np.asarray(W_I_w, dtype=np.float32)
    W_O_w = np.asarray(W_O_w, dtype=np.float32)
    W_S_w = np.asarray(W_S_w, dtype=np.float32)
    W_M_w = np.asarray(W_M_w, dtype=np.float32)

    sched = _build_schedule(src, dst, rel, inv)
    pairs, first, last = _pair_plan(sched['touch'], sched['half_split'],
                                    sched['epad'])
    key = (int(sched['epad']), int(sched['half_split']),
           sched['touch'].tobytes())
    if key not in _CACHE:
        _CACHE[key] = _build_program(pairs, first, last, sched['epad'],
                                     sched['half_split'])
    nc = _CACHE[key]

    vtab = _hilo(rel_emb)
    metarelT = np.zeros((D, 512), np.float32)
    metarelT[:, :N_REL] = metarel_emb.T
    bcast = lambda b: np.tile(np.asarray(b, np.float32)[None, :],
                              (128, 1)).copy()

    shared = {
        "vtab": vtab,
        "metarelT": metarelT,
        "wia_t": W_I_w[:, :D].T.copy(), "woa_t": W_O_w[:, :D].T.copy(),
        "wib_t": W_I_w[:, D:].T.copy(), "wob_t": W_O_w[:, D:].T.copy(),
        "ws_t": W_S_w.T.copy(), "wm_t": W_M_w.T.copy(),
        "bias_i": bcast(W_I_b), "bias_o": bcast(W_O_b),
        "bias_s": bcast(W_S_b), "bias_m": bcast(W_M_b),
        "iota2": np.tile(np.arange(128, dtype=np.float32), (128, 2)).copy(),
    }
    perm = sched['perm']
    in_maps = []
    for c in range(N_CORES):
        rows = np.where(perm[c] >= 0, perm[c], 0)
        relembT = rel_emb[rows].T.copy()
        dspair = _build_dspair(sched['dslot0'][c], sched['dslot1'][c],
                               sched['tileid'][c], pairs)
        in_maps.append(dict(shared,
                            gidx=_wrap16(sched['gidx'][c]),
                            dspair=dspair,
                            ct8=sched['ct8'][c].reshape(125, 8 * SLOTS).copy(),
                            relembT=relembT))

    try:
        res = run_bass_kernel_spmd(nc, in_maps, list(range(N_CORES)))
    except Exception:
        # transient NRT device errors (e.g. a previous crashed process left
        # a core wedged) usually clear on retry
        res = run_bass_kernel_spmd(nc, in_maps, list(range(N_CORES)))
    globals()['_last_results'] = res

    rel_new = np.zeros((N_NODES, D), np.float32)
    for c in range(N_CORES):
        valid = perm[c] >= 0
        rel_new[perm[c][valid]] = res.results[c]["out_rel"][valid]
    meta_new = res.results[0]["out_meta"]
    return rel_new, meta_new
